# revision 1
# baseline (speedup 1.0000x reference)
"""Trainium2 Bass kernel for a fused LSTM cell.

Reference math (B=8192, D=U=1024, all fp32):
    z = x @ Wx + h_tm1 @ Uh + b          # Wx=[W_i|W_f|W_c|W_o], Uh likewise
    i, f = sigmoid(z_i), sigmoid(z_f)
    c = f * c_tm1 + i * tanh(z_c)
    h = sigmoid(z_o) * tanh(c)
    returns (h, c)

Strategy:
  - Data-parallel over 8 NeuronCores: batch 8192 -> 1024 rows/core,
    weights replicated. No collectives.
  - Per core the GEMM is computed transposed: z^T [4096 units, 1024 batch].
    lhsT (stationary) = weight tiles [128k, 128n] in natural [K, N] layout;
    rhs (moving) = host-pretransposed [x|h]^T tiles [128k, 512 batch].
    This puts units on PSUM partitions so the per-unit bias becomes a
    per-partition scalar folded into the ScalarE activation for free.
  - Matmuls run in float32r (fp32 storage, full-rate PE path); everything
    else fp32. Activations (sigmoid/tanh) on ScalarE read PSUM directly;
    gate combine on VectorE.
"""

from contextlib import ExitStack

import numpy as np

import concourse.bass as bass
import concourse.tile as tile
from concourse import bacc, mybir
from concourse.bass_utils import run_bass_kernel_spmd

B, D, U = 8192, 1024, 1024
NCORES = 8
BS = B // NCORES  # per-core batch rows


def build_nc(bs=BS, d=D, u=U, f=512):
    """Build the per-core SPMD Bass program.

    DRAM parameter layouts (host prepares these):
      xh   [KO, 128, bs]    : [x|h]^T, contraction dim on (KO, partition)
      w    [NT, 128, KO, 128]: w[t, p, ko, n] = W_all[ko*128+p, t*128+n]
      bias [128, NT]        : bias[p, t] = b_all[t*128+p]
      ct   [JB, 128, bs]    : c_tm1^T unit-blocks
      h_out/c_out [JB, 128, bs] : h^T / c^T unit-blocks
    """
    kdim = d + u
    KO = kdim // 128   # contraction blocks
    JB = u // 128      # unit blocks per gate
    NT = 4 * u // 128  # total n-tiles (4 gates)
    f = min(f, bs)
    BH = bs // f       # batch chunks of the moving operand

    f32 = mybir.dt.float32
    f32r = mybir.dt.float32r
    SIG = mybir.ActivationFunctionType.Sigmoid
    TANH = mybir.ActivationFunctionType.Tanh

    nc = bacc.Bacc("TRN2", target_bir_lowering=False, debug=False)

    xh = nc.dram_tensor("xh", [KO, BH, 128, f], f32r, kind="ExternalInput").ap()
    # w[j, ko, p, g, n] = W_all[ko*128+p, (g*JB+j)*128+n]
    w = nc.dram_tensor("w", [JB, KO, 128, 4, 128], f32r, kind="ExternalInput").ap()
    bia = nc.dram_tensor("bias", [128, NT], f32, kind="ExternalInput").ap()
    ct = nc.dram_tensor("ct", [JB, 128, bs], f32, kind="ExternalInput").ap()
    ho = nc.dram_tensor("h_out", [JB, 128, bs], f32, kind="ExternalOutput").ap()
    co = nc.dram_tensor("c_out", [JB, 128, bs], f32, kind="ExternalOutput").ap()

    with tile.TileContext(nc) as tc, ExitStack() as ctx:
        xh_pool = ctx.enter_context(tc.tile_pool(name="xh", bufs=1))
        w_pool = ctx.enter_context(tc.tile_pool(name="w", bufs=2 * KO))
        bias_pool = ctx.enter_context(tc.tile_pool(name="bias", bufs=1))
        ct_pool = ctx.enter_context(tc.tile_pool(name="ct", bufs=2))
        gate_pool = ctx.enter_context(tc.tile_pool(name="gates", bufs=2))
        out_pool = ctx.enter_context(tc.tile_pool(name="outs", bufs=2))
        psum_pool = ctx.enter_context(tc.tile_pool(name="psum", bufs=8, space="PSUM"))

        bias_sb = bias_pool.tile([128, NT], f32, tag="bias")
        nc.sync.dma_start(bias_sb[:], bia[:])

        def load_ct(j):
            t = ct_pool.tile([128, bs], f32, tag="ct")
            nc.sync.dma_start(t[:], ct[j])
            return t

        def load_wk(j, ko):
            t = w_pool.tile([128, 4, 128], f32r, tag="wk", name=f"wk_{j}_{ko}")
            nc.sync.dma_start(t[:], w[j, ko])
            return t

        # Startup: j=0's weights interleave with xh so PE starts after
        # the first ~1MB instead of after the full 12MB prefix.
        xh_sb = []
        wk_by_j = {0: []}
        for ko in range(KO):
            wk_by_j[0].append(load_wk(0, ko))
            halves = []
            for bh in range(BH):
                t = xh_pool.tile([128, f], f32r, tag=f"xh{ko}_{bh}", name=f"xh{ko}_{bh}")
                nc.sync.dma_start(t[:], xh[ko, bh])
                halves.append(t)
            xh_sb.append(halves)
        ct_by_j = {0: load_ct(0)}  # not needed until j=0's epilogue

        def epilogue(j, bh, gt, ct_sb, h_out, c_out):
            # Chunk the very last epilogue so its compute overlaps its own
            # output DMA instead of serializing after the final matmul.
            nch = 2 if (j == JB - 1 and bh == BH - 1) else 1
            w = f // nch
            for ci in range(nch):
                gsl = slice(ci * w, (ci + 1) * w)
                bsl = slice(bh * f + ci * w, bh * f + (ci + 1) * w)
                t1 = gate_pool.tile([128, w], f32, tag="t1", name=f"t1_{ci}")
                nc.vector.tensor_mul(t1[:], gt[1][:, gsl], ct_sb[:, bsl])
                t2 = gate_pool.tile([128, w], f32, tag="t2", name=f"t2_{ci}")
                nc.vector.tensor_mul(t2[:], gt[0][:, gsl], gt[2][:, gsl])
                nc.vector.tensor_add(c_out[:, bsl], t1[:], t2[:])
                tct = gate_pool.tile([128, w], f32, tag="tct", name=f"tct_{ci}")
                nc.scalar.activation(tct[:], c_out[:, bsl], TANH)
                nc.vector.tensor_mul(h_out[:, bsl], gt[3][:, gsl], tct[:])
                nc.sync.dma_start(ho[j][:, bsl], h_out[:, bsl])
                nc.sync.dma_start(co[j][:, bsl], c_out[:, bsl])

        def act_gate(j, g, ps):
            gtile = gate_pool.tile([128, f], f32, tag=f"g{g}")
            idx = g * JB + j
            func = TANH if g == 2 else SIG
            nc.scalar.activation(
                gtile[:], ps[:], func, bias=bias_sb[:, idx : idx + 1]
            )
            return gtile

        for j in range(JB):
            # prefetch next block's weights/ct one block ahead
            if j + 1 < JB and (j + 1) not in wk_by_j:
                wk_by_j[j + 1] = [load_wk(j + 1, ko) for ko in range(KO)]
            if j + 1 < JB and (j + 1) not in ct_by_j:
                ct_by_j[j + 1] = load_ct(j + 1)
            wk = wk_by_j.pop(j)
            ct_sb = ct_by_j.pop(j)
            h_out = out_pool.tile([128, bs], f32, tag="h")
            c_out = out_pool.tile([128, bs], f32, tag="c")
            if j <= 1:
                # ko-major: all 8 (g, bh) groups accumulate together so the
                # PE chases the arriving xh/w DMAs instead of waiting for
                # the whole prefix (j=0: xh+weights, j=1: its weight chunks
                # still landing behind the xh stream).
                ps = [
                    [
                        psum_pool.tile(
                            [128, f], f32, tag="ps", name=f"ps_{g}_{bh}"
                        )
                        for bh in range(BH)
                    ]
                    for g in range(4)
                ]
                for ko in range(KO):
                    for bh in range(BH):
                        for g in range(4):
                            nc.tensor.matmul(
                                ps[g][bh][:],
                                lhsT=wk[ko][:, g, :],
                                rhs=xh_sb[ko][bh][:],
                                start=(ko == 0),
                                stop=(ko == KO - 1),
                            )
                for bh in range(BH):
                    gt = [act_gate(j, g, ps[g][bh]) for g in range(4)]
                    epilogue(j, bh, gt, ct_sb, h_out, c_out)
            else:
                for bh in range(BH):
                    bsl = slice(bh * f, (bh + 1) * f)
                    gt = []
                    for g in range(4):
                        psb = psum_pool.tile([128, f], f32, tag="ps")
                        for ko in range(KO):
                            nc.tensor.matmul(
                                psb[:],
                                lhsT=wk[ko][:, g, :],
                                rhs=xh_sb[ko][bh][:],
                                start=(ko == 0),
                                stop=(ko == KO - 1),
                            )
                        gt.append(act_gate(j, g, psb))
                    epilogue(j, bh, gt, ct_sb, h_out, c_out)

    nc.compile()
    return nc


def pack_shared(inputs):
    """Weight + bias device arrays (replicated on every core)."""
    d, u = inputs["W_i"].shape[0], inputs["W_i"].shape[1]
    kdim = d + u
    KO = kdim // 128
    NT = 4 * u // 128
    Wx = np.concatenate(
        [inputs["W_i"], inputs["W_f"], inputs["W_c"], inputs["W_o"]], axis=1
    )
    Uh = np.concatenate(
        [inputs["U_i"], inputs["U_f"], inputs["U_c"], inputs["U_o"]], axis=1
    )
    W_all = np.concatenate([Wx, Uh], axis=0)  # [kdim, 4u]
    JB = u // 128
    # w_dev[j, ko, p, g, n] = W_all[ko*128+p, (g*JB+j)*128+n]
    w_dev = np.ascontiguousarray(
        W_all.reshape(KO, 128, 4, JB, 128).transpose(3, 0, 1, 2, 4)
    ).astype(np.float32)
    b_all = np.concatenate(
        [inputs["b_i"], inputs["b_f"], inputs["b_c"], inputs["b_o"]]
    )  # [4u]
    b_dev = np.ascontiguousarray(b_all.reshape(NT, 128).T).astype(np.float32)
    return w_dev, b_dev


def pack_core(x_i, h_i, c_i, f=512):
    """Per-core shard arrays."""
    bs = x_i.shape[0]
    d, u = x_i.shape[1], h_i.shape[1]
    KO = (d + u) // 128
    JB = u // 128
    f = min(f, bs)
    BH = bs // f
    xh_t = np.concatenate([x_i, h_i], axis=1).T  # [kdim, bs]
    xh_dev = np.ascontiguousarray(
        xh_t.reshape(KO, 128, BH, f).transpose(0, 2, 1, 3)
    ).astype(np.float32)
    ct_dev = np.ascontiguousarray(c_i.T.reshape(JB, 128, bs)).astype(np.float32)
    return xh_dev, ct_dev


_NC_CACHE = {}


def _get_nc():
    key = (BS, D, U)
    if key not in _NC_CACHE:
        _NC_CACHE[key] = build_nc()
    return _NC_CACHE[key]


def _run(inputs, trace=False):
    x = np.asarray(inputs["inputs"], np.float32)
    h = np.asarray(inputs["h_tm1"], np.float32)
    c = np.asarray(inputs["c_tm1"], np.float32)
    w_dev, b_dev = pack_shared(inputs)
    in_maps = []
    for i in range(NCORES):
        sl = slice(i * BS, (i + 1) * BS)
        xh_dev, ct_dev = pack_core(x[sl], h[sl], c[sl])
        in_maps.append({"xh": xh_dev, "w": w_dev, "bias": b_dev, "ct": ct_dev})
    nc = _get_nc()
    res = run_bass_kernel_spmd(nc, in_maps, list(range(NCORES)), trace=trace)
    u = U
    h_full = np.empty((B, u), np.float32)
    c_full = np.empty((B, u), np.float32)
    for i in range(NCORES):
        sl = slice(i * BS, (i + 1) * BS)
        h_full[sl] = res.results[i]["h_out"].reshape(u, BS).T
        c_full[sl] = res.results[i]["c_out"].reshape(u, BS).T
    return (h_full, c_full), res


def kernel(**inputs):
    out, _ = _run(inputs, trace=False)
    return out



# revision 2
# speedup vs baseline: 1.0307x; 1.0307x over previous
"""Trainium2 Bass kernel for a fused LSTM cell.

Reference math (B=8192, D=U=1024, all fp32):
    z = x @ Wx + h_tm1 @ Uh + b          # Wx=[W_i|W_f|W_c|W_o], Uh likewise
    i, f = sigmoid(z_i), sigmoid(z_f)
    c = f * c_tm1 + i * tanh(z_c)
    h = sigmoid(z_o) * tanh(c)
    returns (h, c)

Strategy:
  - Data-parallel over 8 NeuronCores: batch 8192 -> 1024 rows/core,
    weights replicated. No collectives.
  - Per core the GEMM is computed transposed: z^T [4096 units, 1024 batch].
    lhsT (stationary) = weight tiles [128k, 128n]; rhs (moving) =
    host-pretransposed [x|h]^T tiles [128k, 512 batch]. Units on PSUM
    partitions so the per-unit bias folds into the ScalarE activation.
  - GEMM operands in fp16 (PSUM accumulation stays fp32): same 1 col/cycle
    PE rate as fp32r but half the HBM traffic, which makes the j=0 ramp
    PE-bound instead of DMA-bound, and 2-byte weights get the fast
    weight-load path. Quantization error ~2.3e-3 rel (vs 2e-2 budget).
  - j=0 runs ko-major over 6 PSUM groups so the PE chases the arriving
    xh/w stream; its last 2 groups (the c~ gate) run as a second wave so
    j=1 never waits for all 8 PSUM banks at once.
  - Gate order f,i,o,c~ in the steady-state blocks so the final
    dependency chain after the last matmul is as short as possible; the
    very last epilogue is chunked 4x to overlap its own output DMA.
"""

from contextlib import ExitStack

import numpy as np

import concourse.bass as bass
import concourse.tile as tile
from concourse import bacc, mybir
from concourse.bass_utils import run_bass_kernel_spmd

B, D, U = 8192, 1024, 1024
NCORES = 8
BS = B // NCORES  # per-core batch rows
F = 512           # moving-operand cols per matmul (one PSUM bank of fp32)


def build_nc(bs=BS, d=D, u=U):
    """Build the per-core SPMD Bass program.

    DRAM parameter layouts (host prepares these):
      xh   [KO, 128, bs] fp16   : [x|h]^T, contraction on (KO, partition)
      w    [JB, KO2, 128, 2, 4, 128] fp16 :
           w[j,ko2,p,e,g,n] = W_all[(ko2*2+e)*128+p, (g*JB+j)*128+n]
      bias [128, NT] fp32       : bias[p, t] = b_all[t*128+p]
      ct   [JB, 128, bs] fp32   : c_tm1^T unit-blocks
      h_out/c_out [JB, 128, bs] fp32 : h^T / c^T unit-blocks
    """
    kdim = d + u
    KO = kdim // 128    # contraction 128-blocks
    KO2 = KO // 2       # two contraction blocks per weight tile (2KB lines)
    JB = u // 128       # unit blocks per gate
    NT = 4 * u // 128   # total n-tiles (4 gates)
    BH = bs // F        # moving chunks per xh tile

    f32 = mybir.dt.float32
    f16 = mybir.dt.float16
    SIG = mybir.ActivationFunctionType.Sigmoid
    TANH = mybir.ActivationFunctionType.Tanh

    nc = bacc.Bacc("TRN2", target_bir_lowering=False, debug=False)

    xh = nc.dram_tensor("xh", [KO, 128, bs], f16, kind="ExternalInput").ap()
    w = nc.dram_tensor("w", [JB, KO2, 128, 2, 4, 128], f16, kind="ExternalInput").ap()
    bia = nc.dram_tensor("bias", [128, NT], f32, kind="ExternalInput").ap()
    ct = nc.dram_tensor("ct", [JB, 128, bs], f32, kind="ExternalInput").ap()
    ho = nc.dram_tensor("h_out", [JB, 128, bs], f32, kind="ExternalOutput").ap()
    co = nc.dram_tensor("c_out", [JB, 128, bs], f32, kind="ExternalOutput").ap()

    # f gate first so t1 = f*ct can issue early; c~ (tanh, g=2) last so the
    # post-last-matmul chain is just act(c~) -> t2 -> c -> tanh -> h.
    G_ORDER = (1, 0, 3, 2)

    with tile.TileContext(nc) as tc, ExitStack() as ctx:
        xh_pool = ctx.enter_context(tc.tile_pool(name="xh", bufs=1))
        w_pool = ctx.enter_context(tc.tile_pool(name="w", bufs=2 * KO2))
        bias_pool = ctx.enter_context(tc.tile_pool(name="bias", bufs=1))
        ct_pool = ctx.enter_context(tc.tile_pool(name="ct", bufs=2))
        gate_pool = ctx.enter_context(tc.tile_pool(name="gates", bufs=2))
        out_pool = ctx.enter_context(tc.tile_pool(name="outs", bufs=2))
        psum_pool = ctx.enter_context(tc.tile_pool(name="psum", bufs=8, space="PSUM"))

        bias_sb = bias_pool.tile([128, NT], f32, tag="bias")
        nc.sync.dma_start(bias_sb[:], bia[:])

        def load_ct(j):
            t = ct_pool.tile([128, bs], f32, tag="ct")
            nc.sync.dma_start(t[:], ct[j])
            return t

        def load_wk(j, ko2):
            t = w_pool.tile([128, 2, 4, 128], f16, tag="wk", name=f"wk_{j}_{ko2}")
            nc.sync.dma_start(t[:], w[j, ko2])
            return t

        # Startup: interleave j=0 weights with the xh stream so the PE can
        # start after the first ~0.5MB instead of the full prefix.
        ct_by_j = {0: load_ct(0)}
        xh_sb = []
        wk_by_j = {0: []}
        for ko2 in range(KO2):
            wk_by_j[0].append(load_wk(0, ko2))
            for e in range(2):
                t = xh_pool.tile([128, bs], f16, tag=f"xh{2 * ko2 + e}")
                nc.sync.dma_start(t[:], xh[2 * ko2 + e])
                xh_sb.append(t)

        def mm_pair(ps_pair, j, g, ko, wk):
            lhsT = wk[ko // 2][:, ko % 2, g, :]
            for bh in range(BH):
                nc.tensor.matmul(
                    ps_pair[bh][:],
                    lhsT=lhsT,
                    rhs=xh_sb[ko][:, bh * F : (bh + 1) * F],
                    start=(ko == 0),
                    stop=(ko == KO - 1),
                )

        def act_gate(j, g, ps, gtile=None, gsl=slice(0, F)):
            if gtile is None:
                gtile = gate_pool.tile([128, F], f32, tag=f"g{g}")
            idx = g * JB + j
            func = TANH if g == 2 else SIG
            nc.scalar.activation(
                gtile[:, gsl], ps[:, gsl], func, bias=bias_sb[:, idx : idx + 1]
            )
            return gtile

        def epilogue(j, bh, gt, ct_sb, h_out, c_out, nch=1):
            for ci in range(nch):
                wd = F // nch
                gsl = slice(ci * wd, (ci + 1) * wd)
                bsl = slice(bh * F + ci * wd, bh * F + (ci + 1) * wd)
                t1 = gate_pool.tile([128, wd], f32, tag="t1", name=f"t1_{ci}")
                nc.vector.tensor_mul(t1[:], gt[1][:, gsl], ct_sb[:, bsl])
                t2 = gate_pool.tile([128, wd], f32, tag="t2", name=f"t2_{ci}")
                nc.vector.tensor_mul(t2[:], gt[0][:, gsl], gt[2][:, gsl])
                nc.vector.tensor_add(c_out[:, bsl], t1[:], t2[:])
                tct = gate_pool.tile([128, wd], f32, tag="tct", name=f"tct_{ci}")
                nc.scalar.activation(tct[:], c_out[:, bsl], TANH)
                nc.vector.tensor_mul(h_out[:, bsl], gt[3][:, gsl], tct[:])
                nc.sync.dma_start(ho[j][:, bsl], h_out[:, bsl])
                nc.sync.dma_start(co[j][:, bsl], c_out[:, bsl])

        for j in range(JB):
            if j + 1 < JB:
                wk_by_j[j + 1] = [load_wk(j + 1, ko2) for ko2 in range(KO2)]
                ct_by_j[j + 1] = load_ct(j + 1)
            wk = wk_by_j.pop(j)
            ct_sb = ct_by_j.pop(j)
            h_out = out_pool.tile([128, bs], f32, tag="h")
            c_out = out_pool.tile([128, bs], f32, tag="c")
            gt = {}
            if j == 0:
                # ko-major chase in two waves: 6 groups (f,i,o) while xh
                # streams in, then the 2 c~ groups, so j=1's first group
                # only waits on the first wave-1 activations.
                wave1 = G_ORDER[:3]
                ps = {
                    g: [
                        psum_pool.tile([128, F], f32, tag="ps", name=f"ps0_{g}_{bh}")
                        for bh in range(BH)
                    ]
                    for g in G_ORDER
                }
                for ko in range(KO):
                    for g in wave1:
                        mm_pair(ps[g], j, g, ko, wk)
                for ko in range(KO):
                    mm_pair(ps[2], j, 2, ko, wk)
                for g in G_ORDER:
                    gb = []
                    for bh in range(BH):
                        gtile = act_gate(j, g, ps[g][bh])
                        gb.append(gtile)
                    gt[g] = gb
                for bh in range(BH):
                    epilogue(j, bh, [gt[g][bh] for g in range(4)], ct_sb, h_out, c_out)
            else:
                last = j == JB - 1
                for g in G_ORDER:
                    pp = [
                        psum_pool.tile([128, F], f32, tag="ps", name=f"ps_{g}_{bh}")
                        for bh in range(BH)
                    ]
                    for ko in range(KO):
                        mm_pair(pp, j, g, ko, wk)
                    if last and g == 2:
                        # final gate: defer the bh=1 activation into the
                        # chunked epilogue below
                        gt[g] = [act_gate(j, g, pp[0]), pp[1]]
                    else:
                        gt[g] = [act_gate(j, g, pp[bh]) for bh in range(BH)]
                if not last:
                    for bh in range(BH):
                        epilogue(
                            j, bh, [gt[g][bh] for g in range(4)], ct_sb, h_out, c_out
                        )
                else:
                    epilogue(j, 0, [gt[g][0] for g in range(4)], ct_sb, h_out, c_out)
                    # last epilogue: act + combine in 128-col chunks so the
                    # post-matmul serial chain and output DMA overlap.
                    cc = gate_pool.tile([128, F], f32, tag="g2b")
                    for ci in range(4):
                        wd = F // 4
                        gsl = slice(ci * wd, (ci + 1) * wd)
                        act_gate(j, 2, gt[2][1], gtile=cc, gsl=gsl)
                        bsl = slice(F + ci * wd, F + (ci + 1) * wd)
                        t1 = gate_pool.tile([128, wd], f32, tag="t1", name=f"lt1_{ci}")
                        nc.vector.tensor_mul(t1[:], gt[1][1][:, gsl], ct_sb[:, bsl])
                        t2 = gate_pool.tile([128, wd], f32, tag="t2", name=f"lt2_{ci}")
                        nc.vector.tensor_mul(t2[:], gt[0][1][:, gsl], cc[:, gsl])
                        nc.vector.tensor_add(c_out[:, bsl], t1[:], t2[:])
                        tct = gate_pool.tile([128, wd], f32, tag="tct", name=f"ltct_{ci}")
                        nc.scalar.activation(tct[:], c_out[:, bsl], TANH)
                        nc.vector.tensor_mul(h_out[:, bsl], gt[3][1][:, gsl], tct[:])
                        nc.sync.dma_start(ho[j][:, bsl], h_out[:, bsl])
                        nc.sync.dma_start(co[j][:, bsl], c_out[:, bsl])

    nc.compile()
    return nc


def pack_shared(inputs):
    """Weight + bias device arrays (replicated on every core)."""
    d, u = inputs["W_i"].shape[0], inputs["W_i"].shape[1]
    kdim = d + u
    KO = kdim // 128
    KO2 = KO // 2
    NT = 4 * u // 128
    JB = u // 128
    Wx = np.concatenate(
        [inputs["W_i"], inputs["W_f"], inputs["W_c"], inputs["W_o"]], axis=1
    )
    Uh = np.concatenate(
        [inputs["U_i"], inputs["U_f"], inputs["U_c"], inputs["U_o"]], axis=1
    )
    W_all = np.concatenate([Wx, Uh], axis=0)  # [kdim, 4u]
    # w_dev[j, ko2, p, e, g, n] = W_all[(ko2*2+e)*128+p, (g*JB+j)*128+n]
    w_dev = np.ascontiguousarray(
        W_all.reshape(KO2, 2, 128, 4, JB, 128).transpose(4, 0, 2, 1, 3, 5)
    ).astype(np.float16)
    b_all = np.concatenate(
        [inputs["b_i"], inputs["b_f"], inputs["b_c"], inputs["b_o"]]
    )  # [4u]
    b_dev = np.ascontiguousarray(b_all.reshape(NT, 128).T).astype(np.float32)
    return w_dev, b_dev


def pack_core(x_i, h_i, c_i):
    """Per-core shard arrays."""
    bs = x_i.shape[0]
    d, u = x_i.shape[1], h_i.shape[1]
    KO = (d + u) // 128
    JB = u // 128
    xh_t = np.concatenate([x_i, h_i], axis=1).T  # [kdim, bs]
    xh_dev = np.ascontiguousarray(xh_t.reshape(KO, 128, bs)).astype(np.float16)
    ct_dev = np.ascontiguousarray(c_i.T.reshape(JB, 128, bs)).astype(np.float32)
    return xh_dev, ct_dev


_NC_CACHE = {}


def _get_nc():
    key = (BS, D, U)
    if key not in _NC_CACHE:
        _NC_CACHE[key] = build_nc()
    return _NC_CACHE[key]


def _run(inputs, trace=False):
    x = np.asarray(inputs["inputs"], np.float32)
    h = np.asarray(inputs["h_tm1"], np.float32)
    c = np.asarray(inputs["c_tm1"], np.float32)
    w_dev, b_dev = pack_shared(inputs)
    in_maps = []
    for i in range(NCORES):
        sl = slice(i * BS, (i + 1) * BS)
        xh_dev, ct_dev = pack_core(x[sl], h[sl], c[sl])
        in_maps.append({"xh": xh_dev, "w": w_dev, "bias": b_dev, "ct": ct_dev})
    nc = _get_nc()
    res = run_bass_kernel_spmd(nc, in_maps, list(range(NCORES)), trace=trace)
    u = U
    h_full = np.empty((B, u), np.float32)
    c_full = np.empty((B, u), np.float32)
    for i in range(NCORES):
        sl = slice(i * BS, (i + 1) * BS)
        h_full[sl] = res.results[i]["h_out"].reshape(u, BS).T
        c_full[sl] = res.results[i]["c_out"].reshape(u, BS).T
    return (h_full, c_full), res


def kernel(**inputs):
    out, _ = _run(inputs, trace=False)
    return out


# revision 6
# speedup vs baseline: 1.0613x; 1.0297x over previous
"""Trainium2 Bass kernel for a fused LSTM cell.

Reference math (B=8192, D=U=1024, all fp32):
    z = x @ Wx + h_tm1 @ Uh + b          # Wx=[W_i|W_f|W_c|W_o], Uh likewise
    i, f = sigmoid(z_i), sigmoid(z_f)
    c = f * c_tm1 + i * tanh(z_c)
    h = sigmoid(z_o) * tanh(c)
    returns (h, c)

Strategy:
  - Data-parallel over 8 NeuronCores: batch 8192 -> 1024 rows/core,
    weights replicated. No collectives.
  - Per core the GEMM is computed transposed: z^T [4096 units, 1024 batch].
    lhsT (stationary) = weight tiles [128k, 128n]; rhs (moving) =
    host-pretransposed [x|h]^T tiles [128k, 512 batch]. Units on PSUM
    partitions so the per-unit bias folds into the ScalarE activation.
  - GEMM operands in fp16 (PSUM accumulation stays fp32): same 1 col/cycle
    PE rate as fp32r but half the HBM traffic, which makes the j=0 ramp
    PE-bound instead of DMA-bound, and 2-byte weights get the fast
    weight-load path. Quantization error ~2.3e-3 rel (vs 2e-2 budget).
  - j=0 runs ko-major over 6 PSUM groups so the PE chases the arriving
    xh/w stream; its last 2 groups (the c~ gate) run as a second wave so
    j=1 never waits for all 8 PSUM banks at once.
  - Gate order f,i,o,c~ in the steady-state blocks so the final
    dependency chain after the last matmul is as short as possible; the
    very last epilogue is chunked 4x to overlap its own output DMA.
"""

from contextlib import ExitStack

import numpy as np

import concourse.bass as bass
import concourse.tile as tile
from concourse import bacc, mybir
from concourse.bass_utils import run_bass_kernel_spmd

B, D, U = 8192, 1024, 1024
NCORES = 8
BS = B // NCORES  # per-core batch rows
F = 512           # moving-operand cols per matmul (one PSUM bank of fp32)


def build_nc(bs=BS, d=D, u=U):
    """Build the per-core SPMD Bass program.

    DRAM parameter layouts (host prepares these):
      xh   [KO, 128, bs] fp16   : [x|h]^T, contraction on (KO, partition)
      w    [JB, KO2, 128, 2, 4, 128] fp16 :
           w[j,ko2,p,e,g,n] = W_all[(ko2*2+e)*128+p, (g*JB+j)*128+n]
      bias [128, NT] fp32       : bias[p, t] = b_all[t*128+p]
      ct   [JB, 128, bs] fp32   : c_tm1^T unit-blocks
      h_out/c_out [JB, 128, bs] fp32 : h^T / c^T unit-blocks
    """
    kdim = d + u
    KO = kdim // 128    # contraction 128-blocks
    KO2 = KO // 2       # two contraction blocks per weight tile (2KB lines)
    JB = u // 128       # unit blocks per gate
    NT = 4 * u // 128   # total n-tiles (4 gates)
    BH = bs // F        # moving chunks per xh tile

    f32 = mybir.dt.float32
    f16 = mybir.dt.float16
    SIG = mybir.ActivationFunctionType.Sigmoid
    TANH = mybir.ActivationFunctionType.Tanh

    nc = bacc.Bacc("TRN2", target_bir_lowering=False, debug=False)

    xh = nc.dram_tensor("xh", [KO, 128, bs], f16, kind="ExternalInput").ap()
    w = nc.dram_tensor("w", [JB, KO2, 128, 2, 4, 128], f16, kind="ExternalInput").ap()
    bia = nc.dram_tensor("bias", [128, NT], f32, kind="ExternalInput").ap()
    ct = nc.dram_tensor("ct", [JB, 128, bs], f32, kind="ExternalInput").ap()
    ho = nc.dram_tensor("h_out", [JB, 128, bs], f32, kind="ExternalOutput").ap()
    co = nc.dram_tensor("c_out", [JB, 128, bs], f32, kind="ExternalOutput").ap()

    # f gate first so t1 = f*ct can issue early; c~ (tanh, g=2) last so the
    # post-last-matmul chain is just act(c~) -> t2 -> c -> tanh -> h.
    G_ORDER = (1, 0, 3, 2)

    with tile.TileContext(nc) as tc, ExitStack() as ctx:
        xh_pool = ctx.enter_context(tc.tile_pool(name="xh", bufs=1))
        w_pool = ctx.enter_context(tc.tile_pool(name="w", bufs=2 * KO2))
        bias_pool = ctx.enter_context(tc.tile_pool(name="bias", bufs=1))
        ct_pool = ctx.enter_context(tc.tile_pool(name="ct", bufs=2))
        gate_pool = ctx.enter_context(tc.tile_pool(name="gates", bufs=2))
        out_pool = ctx.enter_context(tc.tile_pool(name="outs", bufs=2))
        psum_pool = ctx.enter_context(tc.tile_pool(name="psum", bufs=8, space="PSUM"))

        bias_sb = bias_pool.tile([128, NT], f32, tag="bias")
        nc.sync.dma_start(bias_sb[:], bia[:])

        # HAM warm-up: ~4us of dependency-free matmuls on garbage SBUF while
        # the first real tiles are still in flight, so the PE clock gate is
        # already at 8/8 when the real stream starts.
        warm_sb = gate_pool.tile([128, 128], f16, tag="warm")
        nc.vector.memset(warm_sb[:], 0.0)
        warm_ps = psum_pool.tile([128, F], f32, tag="ps", name="warm_ps")
        for _ in range(70):
            nc.tensor.matmul(
                warm_ps[:, :64], lhsT=warm_sb[:], rhs=warm_sb[:, :64],
                start=True, stop=True,
            )

        def load_ct(j):
            t = ct_pool.tile([128, bs], f32, tag="ct")
            nc.sync.dma_start(t[:], ct[j])
            return t

        def load_wk(j, ko2):
            t = w_pool.tile([128, 2, 4, 128], f16, tag="wk", name=f"wk_{j}_{ko2}")
            nc.sync.dma_start(t[:], w[j, ko2])
            return t

        # Startup: interleave j=0 weights with the xh stream so the PE can
        # start after the first ~0.5MB instead of the full prefix.
        xh_sb = []
        wk_by_j = {0: []}
        ct_by_j = {}
        for ko2 in range(KO2):
            wk_by_j[0].append(load_wk(0, ko2))
            for e in range(2):
                t = xh_pool.tile([128, bs], f16, tag=f"xh{2 * ko2 + e}")
                nc.sync.dma_start(t[:], xh[2 * ko2 + e])
                xh_sb.append(t)
            if ko2 == 0:
                ct_by_j[0] = load_ct(0)

        def mm_pair(ps_pair, j, g, ko, wk):
            lhsT = wk[ko // 2][:, ko % 2, g, :]
            for bh in range(BH):
                nc.tensor.matmul(
                    ps_pair[bh][:],
                    lhsT=lhsT,
                    rhs=xh_sb[ko][:, bh * F : (bh + 1) * F],
                    start=(ko == 0),
                    stop=(ko == KO - 1),
                )

        def act_gate(j, g, ps, gtile=None, gsl=slice(0, F)):
            if gtile is None:
                gtile = gate_pool.tile([128, F], f32, tag=f"g{g}")
            idx = g * JB + j
            func = TANH if g == 2 else SIG
            nc.scalar.activation(
                gtile[:, gsl], ps[:, gsl], func, bias=bias_sb[:, idx : idx + 1]
            )
            return gtile

        def epilogue(j, bh, gt, ct_sb, h_out, c_out, nch=1):
            for ci in range(nch):
                wd = F // nch
                gsl = slice(ci * wd, (ci + 1) * wd)
                bsl = slice(bh * F + ci * wd, bh * F + (ci + 1) * wd)
                t1 = gate_pool.tile([128, wd], f32, tag="t1", name=f"t1_{ci}")
                nc.vector.tensor_mul(t1[:], gt[1][:, gsl], ct_sb[:, bsl])
                t2 = gate_pool.tile([128, wd], f32, tag="t2", name=f"t2_{ci}")
                nc.vector.tensor_mul(t2[:], gt[0][:, gsl], gt[2][:, gsl])
                nc.vector.tensor_add(c_out[:, bsl], t1[:], t2[:])
                tct = gate_pool.tile([128, wd], f32, tag="tct", name=f"tct_{ci}")
                nc.scalar.activation(tct[:], c_out[:, bsl], TANH)
                nc.vector.tensor_mul(h_out[:, bsl], gt[3][:, gsl], tct[:])
                nc.sync.dma_start(ho[j][:, bsl], h_out[:, bsl])
                nc.sync.dma_start(co[j][:, bsl], c_out[:, bsl])

        for j in range(JB):
            if j + 1 < JB:
                wk_by_j[j + 1] = [load_wk(j + 1, ko2) for ko2 in range(KO2)]
                ct_by_j[j + 1] = load_ct(j + 1)
            wk = wk_by_j.pop(j)
            ct_sb = ct_by_j.pop(j)
            h_out = out_pool.tile([128, bs], f32, tag="h")
            c_out = out_pool.tile([128, bs], f32, tag="c")
            gt = {}
            if j == 0:
                # ko-major chase in two waves: 6 groups (f,i,o) while xh
                # streams in, then the 2 c~ groups, so j=1's first group
                # only waits on the first wave-1 activations.
                wave1 = G_ORDER[:3]
                ps = {
                    g: [
                        psum_pool.tile([128, F], f32, tag="ps", name=f"ps0_{g}_{bh}")
                        for bh in range(BH)
                    ]
                    for g in G_ORDER
                }
                for ko in range(KO):
                    for g in wave1:
                        mm_pair(ps[g], j, g, ko, wk)
                for ko in range(KO):
                    mm_pair(ps[2], j, 2, ko, wk)
                for g in G_ORDER:
                    gb = []
                    for bh in range(BH):
                        gtile = act_gate(j, g, ps[g][bh])
                        gb.append(gtile)
                    gt[g] = gb
                for bh in range(BH):
                    epilogue(j, bh, [gt[g][bh] for g in range(4)], ct_sb, h_out, c_out)
            else:
                # bh-major: bh=0's gates+epilogue fully overlap bh=1's
                # matmuls, so only bh=1's final chain trails the last MM.
                last = j == JB - 1
                for bh in range(BH):
                    gtb = {}
                    for g in G_ORDER:
                        psb = psum_pool.tile(
                            [128, F], f32, tag="ps", name=f"ps_{g}_{bh}"
                        )
                        for ko in range(KO):
                            nc.tensor.matmul(
                                psb[:],
                                lhsT=wk[ko // 2][:, ko % 2, g, :],
                                rhs=xh_sb[ko][:, bh * F : (bh + 1) * F],
                                start=(ko == 0),
                                stop=(ko == KO - 1),
                            )
                        if last and bh == BH - 1 and g == 2:
                            gtb[g] = psb  # act deferred into chunked epilogue
                        else:
                            gtb[g] = act_gate(j, g, psb)
                    if not (last and bh == BH - 1):
                        epilogue(j, bh, [gtb[g] for g in range(4)], ct_sb, h_out, c_out)
                    else:
                        # final epilogue: act + combine in 256-col chunks so
                        # the post-matmul chain and output DMA overlap.
                        cc = gate_pool.tile([128, F], f32, tag="g2b")
                        for ci in range(2):
                            wd = F // 2
                            gsl = slice(ci * wd, (ci + 1) * wd)
                            act_gate(j, 2, gtb[2], gtile=cc, gsl=gsl)
                            bsl = slice(bh * F + ci * wd, bh * F + (ci + 1) * wd)
                            t1 = gate_pool.tile([128, wd], f32, tag="t1", name=f"lt1_{ci}")
                            nc.vector.tensor_mul(t1[:], gtb[1][:, gsl], ct_sb[:, bsl])
                            t2 = gate_pool.tile([128, wd], f32, tag="t2", name=f"lt2_{ci}")
                            nc.vector.tensor_mul(t2[:], gtb[0][:, gsl], cc[:, gsl])
                            nc.vector.tensor_add(c_out[:, bsl], t1[:], t2[:])
                            tct = gate_pool.tile([128, wd], f32, tag="tct", name=f"ltct_{ci}")
                            nc.scalar.activation(tct[:], c_out[:, bsl], TANH)
                            nc.vector.tensor_mul(h_out[:, bsl], gtb[3][:, gsl], tct[:])
                            nc.sync.dma_start(ho[j][:, bsl], h_out[:, bsl])
                            nc.sync.dma_start(co[j][:, bsl], c_out[:, bsl])

    nc.compile()
    return nc


def pack_shared(inputs):
    """Weight + bias device arrays (replicated on every core)."""
    d, u = inputs["W_i"].shape[0], inputs["W_i"].shape[1]
    kdim = d + u
    KO = kdim // 128
    KO2 = KO // 2
    NT = 4 * u // 128
    JB = u // 128
    Wx = np.concatenate(
        [inputs["W_i"], inputs["W_f"], inputs["W_c"], inputs["W_o"]], axis=1
    )
    Uh = np.concatenate(
        [inputs["U_i"], inputs["U_f"], inputs["U_c"], inputs["U_o"]], axis=1
    )
    W_all = np.concatenate([Wx, Uh], axis=0)  # [kdim, 4u]
    # w_dev[j, ko2, p, e, g, n] = W_all[(ko2*2+e)*128+p, (g*JB+j)*128+n]
    w_dev = np.ascontiguousarray(
        W_all.reshape(KO2, 2, 128, 4, JB, 128).transpose(4, 0, 2, 1, 3, 5)
    ).astype(np.float16)
    b_all = np.concatenate(
        [inputs["b_i"], inputs["b_f"], inputs["b_c"], inputs["b_o"]]
    )  # [4u]
    b_dev = np.ascontiguousarray(b_all.reshape(NT, 128).T).astype(np.float32)
    return w_dev, b_dev


def pack_core(x_i, h_i, c_i):
    """Per-core shard arrays."""
    bs = x_i.shape[0]
    d, u = x_i.shape[1], h_i.shape[1]
    KO = (d + u) // 128
    JB = u // 128
    xh_t = np.concatenate([x_i, h_i], axis=1).T  # [kdim, bs]
    xh_dev = np.ascontiguousarray(xh_t.reshape(KO, 128, bs)).astype(np.float16)
    ct_dev = np.ascontiguousarray(c_i.T.reshape(JB, 128, bs)).astype(np.float32)
    return xh_dev, ct_dev


_NC_CACHE = {}


def _get_nc():
    key = (BS, D, U)
    if key not in _NC_CACHE:
        _NC_CACHE[key] = build_nc()
    return _NC_CACHE[key]


def _run(inputs, trace=False):
    x = np.asarray(inputs["inputs"], np.float32)
    h = np.asarray(inputs["h_tm1"], np.float32)
    c = np.asarray(inputs["c_tm1"], np.float32)
    w_dev, b_dev = pack_shared(inputs)
    in_maps = []
    for i in range(NCORES):
        sl = slice(i * BS, (i + 1) * BS)
        xh_dev, ct_dev = pack_core(x[sl], h[sl], c[sl])
        in_maps.append({"xh": xh_dev, "w": w_dev, "bias": b_dev, "ct": ct_dev})
    nc = _get_nc()
    res = run_bass_kernel_spmd(nc, in_maps, list(range(NCORES)), trace=trace)
    u = U
    h_full = np.empty((B, u), np.float32)
    c_full = np.empty((B, u), np.float32)
    for i in range(NCORES):
        sl = slice(i * BS, (i + 1) * BS)
        h_full[sl] = res.results[i]["h_out"].reshape(u, BS).T
        c_full[sl] = res.results[i]["c_out"].reshape(u, BS).T
    return (h_full, c_full), res


def kernel(**inputs):
    out, _ = _run(inputs, trace=False)
    return out
